# revision 6
# baseline (speedup 1.0000x reference)
"""CNOT permutation kernel for Trainium2 (Bass), 8-core data parallel. v3.

v1 architecture (loads on sync HWDGE ring, stores on scalar ring, NBUF-deep
slab pipeline) + head/tail slab piece-split: the first and last swap slabs
are moved as two 2 MiB [128, 4096] pieces instead of one 4 MiB [128, 8192]
slab, so the first store can start after ~5 us (half the data) and the
final store drains in pieces.  Every load still covers all 128 partitions
(= all 16 SDMA engines, one sem-inc each), so the cumulative
wait_ge(load_sem, 16*k) <=> "first k loads fully landed" invariant is
airtight.

Slab layout reminder, per 4 MiB swap slab viewed flat as [c:64][d:2][e:8192]:
full-slab tile [128, 8192] puts chunk (c, d) on partition p = 2c + d
(32 KiB per partition = 1 chunk); half-slab tile [128, 4096] puts half of
chunk k = (c, d), d = k % 2, on partitions 4c + 2d + {0, 1} (16 KiB per
partition = half a chunk), so the d-swap store uses partition stride 4.
"""

import numpy as np

import concourse.bass as bass
import concourse.mybir as mybir
from concourse.bass_utils import run_bass_kernel_spmd

NUM_QUBITS = 24
DIM = 1 << NUM_QUBITS
BATCH = 16
N_CORES = 8
ROWS = BATCH // N_CORES  # 2 rows per core
C2 = NUM_QUBITS - 3 - 1  # 20
T2 = NUM_QUBITS - 10 - 1  # 13
CBIT = 1 << C2  # 1048576 elements (4 MiB)
TBIT = 1 << T2  # 8192 elements (32 KiB)
BLK = 2 * CBIT  # control-bit period
NBLK = ROWS * DIM // BLK  # 16 blocks in the fused per-core space

P = 128
FREE = CBIT // P  # 8192: slab is [128, 8192] f32 = 4 MiB
HFREE = FREE // 2  # 4096: half-slab piece [128, 4096] = 2 MiB
NBUF = 6

_cache = {}


def _emit_bounce(nc, src, dst, slabs, split_head_tail):
    """Loads on sync / stores on scalar, NBUF-deep pipeline over slabs.

    Each unit is (slab_index, base, swap, piece) where piece is None for a
    full 4 MiB slab or 0/1 for the 2 MiB halves of a split slab.  Units of
    one slab share that slab's SBUF buffer (disjoint free-dim halves).
    load_counts[u] = number of load DMAs issued up to and including unit u;
    store_counts[s] = number of store DMAs for all slabs <= s (for the WAR
    buffer-reuse gate, which stays slab-granular).
    """
    units = []
    for si, (base, swap) in enumerate(slabs):
        if split_head_tail and swap and si in (0, len(slabs) - 1):
            units.append((si, base, swap, 0))
            units.append((si, base, swap, 1))
        else:
            units.append((si, base, swap, None))

    n_slabs = len(slabs)
    # store DMAs per unit: full slab -> 2; half piece -> 4
    stores_per_unit = [4 if u[3] is not None else 2 for u in units]
    load_prefix = list(range(1, len(units) + 1))  # 1 load DMA per unit
    store_prefix = np.cumsum(stores_per_unit).tolist()

    with (
        nc.sbuf_tensor("tiles", [P, NBUF * FREE], mybir.dt.float32) as tiles,
        nc.semaphore("load_sem") as load_sem,
        nc.semaphore("store_sem") as store_sem,
        nc.Block() as block,
    ):

        def tile_view(si, piece):
            sl = tiles[:, (si % NBUF) * FREE : (si % NBUF + 1) * FREE]
            if piece is None:
                return sl
            return sl[:, piece * HFREE : (piece + 1) * HFREE]

        # stores for all slabs < si complete (WAR gate for buffer reuse)
        def stores_done_through(si):
            total = 0
            for u, (sj, _b, _s, _p) in enumerate(units):
                if sj <= si:
                    total += stores_per_unit[u]
            return total

        @block.sync
        def _(sync):
            for u, (si, base, swap, piece) in enumerate(units):
                if si >= NBUF and (piece is None or piece == 0):
                    sync.wait_ge(store_sem, 16 * stores_done_through(si - NBUF))
                off = 0 if piece in (None, 0) else CBIT // 2
                size = CBIT if piece is None else CBIT // 2
                sync.dma_start(
                    out=tile_view(si, piece),
                    in_=bass.AP(src, base + off, [[1, size]]),
                ).then_inc(load_sem, 16)

        @block.scalar
        def _(scalar):
            for u, (si, base, swap, piece) in enumerate(units):
                scalar.wait_ge(load_sem, 16 * load_prefix[u])
                t = tile_view(si, piece)
                if piece is not None:
                    # half-slab: partition p holds half of chunk k = p // 2,
                    # d = k % 2, half = p % 2; piece covers c in
                    # [piece*32, piece*32+32)
                    pbase = base + piece * (CBIT // 2)
                    hb = TBIT // 2  # 4096 elements per partition line
                    for d0 in range(2):  # data d value
                        for h in range(2):  # half of the chunk
                            scalar.dma_start(
                                out=bass.AP(
                                    dst,
                                    pbase + (1 - d0) * TBIT + h * hb,
                                    [[2 * TBIT, 32], [1, hb]],
                                ),
                                in_=t[2 * d0 + h :: 4, :],
                            ).then_inc(store_sem, 16)
                elif swap:
                    # partition p holds chunk (c, d) with p = 2c + d
                    scalar.dma_start(
                        out=bass.AP(dst, base + TBIT, [[2 * TBIT, P // 2], [1, TBIT]]),
                        in_=t[0::2, :],  # d=0 data -> d=1 positions
                    ).then_inc(store_sem, 16)
                    scalar.dma_start(
                        out=bass.AP(dst, base, [[2 * TBIT, P // 2], [1, TBIT]]),
                        in_=t[1::2, :],  # d=1 data -> d=0 positions
                    ).then_inc(store_sem, 16)
                else:
                    scalar.dma_start(
                        out=bass.AP(dst, base, [[1, CBIT // 2]]),
                        in_=t[0 : P // 2, :],
                    ).then_inc(store_sem, 16)
                    scalar.dma_start(
                        out=bass.AP(dst, base + CBIT // 2, [[1, CBIT // 2]]),
                        in_=t[P // 2 :, :],
                    ).then_inc(store_sem, 16)
            scalar.wait_ge(store_sem, 16 * store_prefix[-1])


def _build_nc(inplace):
    nc = bass.Bass(target_bir_lowering=False)
    out = nc.dram_tensor("out", (ROWS, DIM), mybir.dt.float32, kind="ExternalOutput")
    if inplace:
        slabs = [(b * BLK + CBIT, True) for b in range(NBLK)]
        _emit_bounce(nc, out, out, slabs, split_head_tail=True)
    else:
        st = nc.dram_tensor(
            "state", (ROWS, DIM), mybir.dt.float32, kind="ExternalInput"
        )
        slabs = []
        for b in range(NBLK):
            slabs.append((b * BLK, False))
            slabs.append((b * BLK + CBIT, True))
        _emit_bounce(nc, st, out, slabs, split_head_tail=False)
    if not nc.is_finalized():
        nc.finalize()
    return nc


def _get_nc(inplace):
    key = ("ip" if inplace else "fc",)
    if key not in _cache:
        _cache[key] = _build_nc(inplace)
    return _cache[key]


def _run_donated(nc, state):
    """Run `nc` via PJRT shard_map over 8 cores, donating the input state as
    the initial content of the (aliased) output buffer — the same donation
    mechanism run_bass_via_pjrt uses for its zero-filled outputs."""
    import jax

    try:
        from jax.experimental.shard_map import shard_map
    except ImportError:  # moved in newer jax
        from jax import shard_map
    from jax.sharding import Mesh, PartitionSpec

    from concourse.bass2jax import (
        _bass_exec_p,
        install_neuronx_cc_hook,
        partition_id_tensor,
    )

    install_neuronx_cc_hook()

    try:
        shaped_array = jax.core.ShapedArray
    except AttributeError:  # moved in newer jax
        from jax._src.core import ShapedArray as shaped_array

    out_names, out_avals = [], []
    for alloc in nc.m.functions[0].allocations:
        if (
            isinstance(alloc, mybir.MemoryLocationSet)
            and alloc.kind == "ExternalOutput"
        ):
            out_names.append(alloc.memorylocations[0].name)
            out_avals.append(
                shaped_array(tuple(alloc.tensor_shape), mybir.dt.np(alloc.dtype))
            )
    partition_name = nc.partition_id_tensor.name if nc.partition_id_tensor else None
    in_names = list(out_names)
    if partition_name is not None:
        in_names.append(partition_name)

    if "donated_fn" not in _cache:

        def _body(buf):
            operands = [buf]
            if partition_name is not None:
                operands.append(partition_id_tensor())
            outs = _bass_exec_p.bind(
                *operands,
                out_avals=tuple(out_avals),
                in_names=tuple(in_names),
                out_names=tuple(out_names),
                lowering_input_output_aliases=(),
                sim_require_finite=True,
                sim_require_nnan=True,
                nc=nc,
            )
            return outs[0]

        devices = jax.devices()[:N_CORES]
        mesh = Mesh(np.asarray(devices), ("core",))
        _cache["donated_fn"] = jax.jit(
            shard_map(
                _body,
                mesh=mesh,
                in_specs=(PartitionSpec("core"),),
                out_specs=PartitionSpec("core"),
                check_rep=False,
            ),
            donate_argnums=(0,),
            keep_unused=True,
        )

    out = _cache["donated_fn"](state)
    return np.asarray(out)


def _sample_ok(state, out, rng, k=2048):
    """Spot-check out[b, j] == state[b, j ^ (1<<13) if bit20(j) else j]."""
    b = rng.integers(0, BATCH, size=k)
    j = rng.integers(0, DIM, size=k)
    src = np.where((j >> C2) & 1 == 1, j ^ TBIT, j)
    return np.array_equal(out[b, j], state[b, src])


def kernel(state, control=3, target=10, num_qubits=24, **_):
    state = np.ascontiguousarray(np.asarray(state, dtype=np.float32))
    assert state.shape == (BATCH, DIM), state.shape
    assert int(control) == 3 and int(target) == 10 and int(num_qubits) == 24

    rng = np.random.default_rng(0)
    # two attempts: donation failures can be transient (buffer held elsewhere)
    for _attempt in range(2):
        try:
            out = _run_donated(_get_nc(inplace=True), state)
            if _sample_ok(state, out, rng):
                return out
        except Exception:
            pass

    # Fallback: full-copy kernel through run_bass_kernel_spmd.
    nc = _get_nc(inplace=False)
    in_maps = [{"state": state[c * ROWS : (c + 1) * ROWS]} for c in range(N_CORES)]
    res = run_bass_kernel_spmd(nc, in_maps, core_ids=list(range(N_CORES)))
    return np.concatenate([r["out"] for r in res.results], axis=0)
